# revision 1
# baseline (speedup 1.0000x reference)
"""Trainium2 Bass kernel for the NeuralMemory (scatter_memory) problem.

Math (per batch b, derived in closed form from the reference):
  keys/vals/q = l2norm_over_T(silu(x @ W.T))          (feature-major)
  a    = W1 @ keys^T ; h = silu(a) ; sp = silu'(a)
  cd   = coeff_eff[t] * ((W2 @ h^T) - vals^T)          (coeff_eff = coeff*2/(E*B))
  ce   = (W2^T @ cd) * sp
  W1f^T = decay*W1^T + sum_t keys[t] ce[t]^T           (T-contraction matmul)
  W2f^T = decay*W2^T + sum_t h[t] cd[t]^T
  b1f  = sum_t ce[t] ; b2f = sum_t cd[t]
  out  = W2f @ silu(W1f @ q^T + b1f) + b2f

Layout: "fm" = feature-major packed (128, 512): partition p = e + 64*j,
column t' with t = t' + 512*j.  T-major chunks via PE transpose-mode against
an identity.  The l2norm scales for keys and q are folded into downstream
matmul stationaries, so the raw silu outputs feed the transposes directly.
One batch per NeuronCore (8 cores).
"""

import os

import numpy as np

import concourse.bacc as bacc
import concourse.mybir as mybir
from concourse.tile import TileContext
from concourse.bass_utils import run_bass_kernel_spmd

ALPHA, ETA, THETA = 0.999, 0.6, 0.05
B, T, E, H = 8, 1024, 64, 64
FP = mybir.dt.float32
I32 = mybir.dt.int32
AF = mybir.ActivationFunctionType
ALU = mybir.AluOpType
MAGIC = 0x5F3759DF

_NC_CACHE = {}

# blobA columns: x (0:512) | kwT vwT qwT w1T w2T w2d (512:896), all dup'd
# blobB columns: I128 (0:128) | coeff_bc (128:640) | dW1T (640:704, rows 0:64)
#                | dW2T (704:768, rows 0:64)
BLOBA_COLS = 896
BLOBB_COLS = 768


def _emit_dual(nc, psum, lhsT_dup, rhs_fm, start=True, stop=True):
    nc.tensor.matmul(psum[0:64, :], lhsT_dup[0:64, :], rhs_fm[0:64, :],
                     start=start, stop=stop)
    nc.tensor.matmul(psum[64:128, :], lhsT_dup[64:128, :], rhs_fm[64:128, :],
                     start=start, stop=stop)


def build_nc(finalize=True, bench_iters=1, ablate=()):
    ablate = set(ablate) | set(
        a for a in os.environ.get("KERNEL_ABLATE", "").split(",") if a)
    nc = bacc.Bacc("TRN2", target_bir_lowering=False, debug=False)

    blobA_d = nc.declare_dram_parameter("blobA", [128, BLOBA_COLS], FP,
                                        isOutput=False)
    blobB_d = nc.declare_dram_parameter("blobB", [128, BLOBB_COLS], FP,
                                        isOutput=False)
    out_d = nc.declare_dram_parameter("outp", [128, 512], FP, isOutput=True)

    with TileContext(nc) as tc:
        with (
            tc.tile_pool(name="persist", bufs=1) as pp,
            tc.tile_pool(name="rot", bufs=2) as rot,
            tc.tile_pool(name="small", bufs=1) as sm,
            tc.tile_pool(name="psmm", bufs=3, space="PSUM") as psmm,
            tc.tile_pool(name="pstr", bufs=3, space="PSUM") as pstr,
            tc.tile_pool(name="psacc", bufs=1, space="PSUM") as psacc,
        ):
            blobA = pp.tile([128, BLOBA_COLS], FP, tag="blobA", name="blobA")
            nc.sync.dma_start(out=blobA[:, :], in_=blobA_d[:, :])
            blobB = pp.tile([128, BLOBB_COLS], FP, tag="blobB", name="blobB")
            nc.sync.dma_start(out=blobB[:, :], in_=blobB_d[:, :])

            x_sb = blobA[:, 0:512]
            wt = {}
            for i, nm in enumerate(["kwT", "vwT", "qwT", "w1T", "w2T", "w2d"]):
                wt[nm] = blobA[:, 512 + 64 * i:576 + 64 * i]
            I128 = blobB[:, 0:128]
            coeff_bc = blobB[:, 128:640]
            dW1T = blobB[0:64, 640:704]
            dW2T = blobB[0:64, 704:768]

            # small constants (no DMA deps)
            magic = sm.tile([64, 1], I32, tag="magic", name="magic")
            nc.vector.memset(magic[:, :], MAGIC)
            wrow = pp.tile([128, 512], FP, tag="wrow", name="wrow")
            nc.gpsimd.memset(wrow[:, :], 0.0)
            warm_lhs = sm.tile([128, 1], FP, tag="warm_lhs", name="warm_lhs")
            nc.vector.memset(warm_lhs[:, :], 0.0)

            out_sb = pp.tile([128, 512], FP, tag="out_sb", name="out_sb")

            # ---- PE warm-up during the input DMA (ramps the p-state) ----
            pswarm = psmm.tile([128, 512], FP, tag="mm", name="mm")
            for _ in range(0 if "warm" in ablate else 4):
                nc.tensor.matmul(pswarm[0:1, :], warm_lhs[:, 0:1], wrow[:, :],
                                 start=True, stop=True)

            import contextlib
            _loop = contextlib.ExitStack()
            if bench_iters > 1:
                _loop.enter_context(tc.For_i(0, bench_iters, 1))

            def rsqrt64(s2, nm):
                """1/sqrt(s2) on DVE via fast-inverse-sqrt + 3 Newton steps."""
                s2h = sm.tile([64, 1], FP, tag=f"s2h_{nm}", name=f"s2h_{nm}")
                nc.vector.tensor_scalar_mul(s2h[:, :], s2[:, :], 0.5)
                sh1 = sm.tile([64, 1], I32, tag=f"sh1_{nm}", name=f"sh1_{nm}")
                nc.vector.tensor_scalar(
                    out=sh1[:, :], in0=s2[:, :].bitcast(I32), scalar1=1,
                    scalar2=None, op0=ALU.arith_shift_right)
                y0 = sm.tile([64, 1], I32, tag=f"y0_{nm}", name=f"y0_{nm}")
                nc.vector.tensor_sub(y0[:, :], magic[:, :], sh1[:, :])
                y = y0[:, :].bitcast(FP)
                yn = None
                for it in range(3):
                    yy = sm.tile([64, 1], FP, tag=f"yy{it}_{nm}",
                                 name=f"yy{it}_{nm}")
                    nc.vector.tensor_mul(yy[:, :], y, y)
                    nc.vector.tensor_mul(yy[:, :], yy[:, :], s2h[:, :])
                    nc.vector.tensor_scalar(
                        out=yy[:, :], in0=yy[:, :], scalar1=-1.0, scalar2=1.5,
                        op0=ALU.mult, op1=ALU.add)
                    yn = sm.tile([64, 1], FP, tag=f"yn{it}_{nm}",
                                 name=f"yn{it}_{nm}")
                    nc.vector.tensor_mul(yn[:, :], y, yy[:, :])
                    y = yn[:, :]
                return yn

            # ---- phase 1: silu + (folded) l2norm scales ----
            def stream(wname, nm):
                ps = psmm.tile([128, 512], FP, tag="mm", name="mm")
                _emit_dual(nc, ps, wt[wname], x_sb)
                sig = rot.tile([128, 512], FP, tag="sig", name="sig")
                nc.scalar.activation(sig[:, :], ps[:, :], AF.Sigmoid)
                sil = pp.tile([128, 512], FP, tag=f"sil_{nm}", name=f"sil_{nm}")
                nc.vector.tensor_mul(sil[:, :], ps[:, :], sig[:, :])
                sq = rot.tile([128, 512], FP, tag="sq", name="sq")
                sums = sm.tile([128, 1], FP, tag=f"sums_{nm}",
                               name=f"sums_{nm}")
                nc.scalar.activation(sq[:, :], sil[:, :], AF.Square,
                                     accum_out=sums[:, :])
                shh = sm.tile([64, 1], FP, tag=f"shh_{nm}", name=f"shh_{nm}")
                nc.vector.tensor_copy(shh[:, :], sums[64:128, :])
                s2 = sm.tile([64, 1], FP, tag=f"s2_{nm}", name=f"s2_{nm}")
                nc.vector.tensor_add(s2[:, :], sums[0:64, :], shh[:, :])
                rs = rsqrt64(s2, nm)
                return sil, rs

            silk, rs_k = stream("kwT", "k")
            silv, rs_v = stream("vwT", "v")
            silq, rs_q = stream("qwT", "q")

            # vals needs the materialized normalized values
            rs_v128 = sm.tile([128, 1], FP, tag="rs_v128", name="rs_v128")
            nc.vector.tensor_copy(rs_v128[0:64, :], rs_v[:, :])
            nc.vector.tensor_copy(rs_v128[64:128, :], rs_v[:, :])
            vals_fm = pp.tile([128, 512], FP, tag="vals_fm", name="vals_fm")
            nc.vector.tensor_scalar_mul(vals_fm[:, :], silv[:, :],
                                        rs_v128[:, :])

            # keys scale folds into the W1 stationary
            rs_k128 = sm.tile([128, 1], FP, tag="rs_k128", name="rs_k128")
            nc.vector.tensor_copy(rs_k128[0:64, :], rs_k[:, :])
            nc.vector.tensor_copy(rs_k128[64:128, :], rs_k[:, :])
            w1Ts = sm.tile([128, 64], FP, tag="w1Ts", name="w1Ts")
            nc.vector.tensor_scalar_mul(w1Ts[:, :], wt["w1T"], rs_k128[:, :])

            # ---- phase 2: a, h, sp ----
            psA = psmm.tile([128, 512], FP, tag="mm", name="mm")
            _emit_dual(nc, psA, w1Ts[:, :], silk[:, :])
            sigA = rot.tile([128, 512], FP, tag="sig", name="sig")
            nc.scalar.activation(sigA[:, :], psA[:, :], AF.Sigmoid)
            h_fm = pp.tile([128, 512], FP, tag="h_fm", name="h_fm")
            nc.vector.tensor_mul(h_fm[:, :], psA[:, :], sigA[:, :])
            # sp = sigA * ((a + 1) - h)
            t2 = rot.tile([128, 512], FP, tag="t2", name="t2")
            nc.vector.scalar_tensor_tensor(
                out=t2[:, :], in0=psA[:, :], scalar=1.0, in1=h_fm[:, :],
                op0=ALU.add, op1=ALU.subtract)
            sp_fm = pp.tile([128, 512], FP, tag="sp_fm", name="sp_fm")
            nc.vector.tensor_mul(sp_fm[:, :], sigA[:, :], t2[:, :])

            # ---- phase 3: cd, ce ----
            psP = psmm.tile([128, 512], FP, tag="mm", name="mm")
            _emit_dual(nc, psP, wt["w2T"], h_fm[:, :])
            dr = rot.tile([128, 512], FP, tag="dr", name="dr")
            nc.vector.tensor_sub(dr[:, :], psP[:, :], vals_fm[:, :])
            cd_fm = pp.tile([128, 512], FP, tag="cd_fm", name="cd_fm")
            nc.vector.tensor_mul(cd_fm[:, :], dr[:, :], coeff_bc)

            psE = psmm.tile([128, 512], FP, tag="mm", name="mm")
            _emit_dual(nc, psE, wt["w2d"], cd_fm[:, :])
            ce_fm = pp.tile([128, 512], FP, tag="ce_fm", name="ce_fm")
            nc.vector.tensor_mul(ce_fm[:, :], psE[:, :], sp_fm[:, :])

            # ---- bias cols: b1f = sum_t ce, b2f = sum_t cd (DVE reductions) --
            def bias_col(src_fm, nm):
                sums = sm.tile([128, 1], FP, tag=f"bs_{nm}", name=f"bs_{nm}")
                nc.vector.reduce_sum(sums[:, :], src_fm[:, :],
                                     axis=mybir.AxisListType.X)
                shh = sm.tile([64, 1], FP, tag=f"bsh_{nm}", name=f"bsh_{nm}")
                nc.vector.tensor_copy(shh[:, :], sums[64:128, :])
                col = sm.tile([128, 1], FP, tag=f"bcol_{nm}",
                              name=f"bcol_{nm}")
                nc.vector.tensor_add(col[0:64, :], sums[0:64, :], shh[:, :])
                nc.vector.tensor_copy(col[64:128, :], col[0:64, :])
                return col

            b1c = bias_col(ce_fm, "b1")
            b2c = bias_col(cd_fm, "b2")

            # ---- phase 4: transposes to T-major chunks (PE transpose mode) --
            fused_b = bool(os.environ.get("KERNEL_FUSED_B"))
            srcs = (("k", silk, "act"), ("h", h_fm, "act"),
                    ("e", ce_fm, "dve"), ("d", cd_fm, "dve"))
            if fused_b:
                # kh_tr[cc] = [keys_cc | h_cc], ed_tr[cc] = [ce_cc | cd_cc]
                kh_tr = [pp.tile([128, 128], FP, tag=f"kh_tr{cc}",
                                 name=f"kh_tr{cc}") for cc in range(8)]
                ed_tr = [pp.tile([128, 128], FP, tag=f"ed_tr{cc}",
                                 name=f"ed_tr{cc}") for cc in range(8)]
                for nm, src_fm, eng in srcs:
                    col = 0 if nm in ("k", "e") else 64
                    grp = kh_tr if nm in ("k", "h") else ed_tr
                    for c in range(4):
                        if "tr" in ablate:
                            continue
                        ps = pstr.tile([128, 128], FP, tag="tr", name="tr")
                        nc.tensor.transpose(
                            ps[:, :], src_fm[:, 128 * c:128 * (c + 1)], I128)
                        cp = (nc.scalar.copy if eng == "act"
                              else nc.vector.tensor_copy)
                        cp(grp[c][:, col:col + 64], ps[:, 0:64])
                        cp(grp[c + 4][:, col:col + 64], ps[:, 64:128])
            else:
                trs = {}
                for nm, src_fm, eng in srcs:
                    tiles = []
                    for c in range(4):
                        dst = pp.tile([128, 128], FP, tag=f"{nm}_tr{c}",
                                      name=f"{nm}_tr{c}")
                        if "tr" not in ablate:
                            ps = pstr.tile([128, 128], FP, tag="tr", name="tr")
                            nc.tensor.transpose(
                                ps[:, :], src_fm[:, 128 * c:128 * (c + 1)],
                                I128)
                            cp = (nc.scalar.copy if eng == "act"
                                  else nc.vector.tensor_copy)
                            cp(dst[:, :], ps[:, :])
                        tiles.append(dst)
                    trs[nm] = tiles

            # ---- phase 5: T-contraction ----
            # Q11' = sum_t keys_raw[t] ce[t]^T (e x h); Q22 = sum_t h[t] cd[t]^T
            if fused_b:
                psBf = psacc.tile([128, 128], FP, tag="psBf", name="psBf")
                for cc in (range(0) if "B" in ablate else range(8)):
                    nc.tensor.matmul(psBf[:, :], kh_tr[cc][:, :],
                                     ed_tr[cc][:, :], start=(cc == 0),
                                     stop=(cc == 7))
                q11 = psBf[0:64, 0:64]
                q22 = psBf[64:128, 64:128]
            else:
                psB1 = psacc.tile([64, 64], FP, tag="psB1", name="psB1")
                psB2 = psacc.tile([128, 64], FP, tag="psB2", name="psB2")
                for cc in (range(0) if "B" in ablate else range(8)):
                    c, base = cc % 4, 64 * (cc // 4)
                    nc.tensor.matmul(psB1[:, :],
                                     trs["k"][c][:, base:base + 64],
                                     trs["e"][c][:, base:base + 64],
                                     start=(cc == 0), stop=(cc == 7),
                                     skip_group_check=True)
                    nc.tensor.matmul(psB2[64:128, :],
                                     trs["h"][c][:, base:base + 64],
                                     trs["d"][c][:, base:base + 64],
                                     start=(cc == 0), stop=(cc == 7),
                                     skip_group_check=True)
                q11 = psB1[:, :]
                q22 = psB2[64:128, :]

            # ---- phase 6: final fast weights ----
            # W1fT = (rs_k*rs_q)[e]*Q11' + rs_q[e]*decay*W1T; W2fT = Q22+decay*W2T
            if "B" in ablate:
                # timing-ablation fallback weights
                w1fT = wt["w1T"]
                w2fT = wt["w2T"]
            skq = sm.tile([64, 1], FP, tag="skq", name="skq")
            if "B" not in ablate:
                nc.vector.tensor_mul(skq[:, :], rs_k[:, :], rs_q[:, :])
                dW1q = sm.tile([64, 64], FP, tag="dW1q", name="dW1q")
                nc.vector.tensor_scalar_mul(dW1q[:, :], dW1T, rs_q[:, :])
                w1fT = pp.tile([128, 64], FP, tag="w1fT", name="w1fT")
                nc.vector.scalar_tensor_tensor(
                    out=w1fT[0:64, :], in0=q11, scalar=skq[:, :],
                    in1=dW1q[:, :], op0=ALU.mult, op1=ALU.add)
                nc.vector.scalar_tensor_tensor(
                    out=w1fT[64:128, :], in0=q11, scalar=skq[:, :],
                    in1=dW1q[:, :], op0=ALU.mult, op1=ALU.add)
                w2fT = pp.tile([128, 64], FP, tag="w2fT", name="w2fT")
                nc.vector.scalar_tensor_tensor(
                    out=w2fT[0:64, :], in0=q22, scalar=1.0,
                    in1=dW2T, op0=ALU.mult, op1=ALU.add)
                nc.vector.scalar_tensor_tensor(
                    out=w2fT[64:128, :], in0=q22, scalar=1.0,
                    in1=dW2T, op0=ALU.mult, op1=ALU.add)

            # ---- phase 7: retrieval ----
            psR1 = psmm.tile([128, 512], FP, tag="mm", name="mm")
            _emit_dual(nc, psR1, w1fT[:, :], silq[:, :])
            sigR = rot.tile([128, 512], FP, tag="sig", name="sig")
            nc.scalar.activation(sigR[:, :], psR1[:, :], AF.Sigmoid,
                                 bias=b1c[:, :])
            h2_fm = pp.tile([128, 512], FP, tag="h2_fm", name="h2_fm")
            nc.vector.scalar_tensor_tensor(
                out=h2_fm[:, :], in0=psR1[:, :], scalar=b1c[:, :],
                in1=sigR[:, :], op0=ALU.add, op1=ALU.mult)

            psR2 = psmm.tile([128, 512], FP, tag="mm", name="mm")
            _emit_dual(nc, psR2, w2fT[:, :], h2_fm[:, :])
            nc.scalar.activation(out_sb[:, :], psR2[:, :], AF.Identity,
                                 bias=b2c[:, :])

            _loop.close()
            nc.sync.dma_start(out=out_d[:, :], in_=out_sb[:, :])

    if finalize:
        nc.finalize()
    return nc


def _get_nc():
    if "nc" not in _NC_CACHE:
        _NC_CACHE["nc"] = build_nc()
    return _NC_CACHE["nc"]


def _host_inputs(x, Kw, Qw, Vw, W1, b1, W2, b2):
    x = np.asarray(x, np.float32)
    Kw = np.asarray(Kw, np.float32)
    Qw = np.asarray(Qw, np.float32)
    Vw = np.asarray(Vw, np.float32)
    W1 = np.asarray(W1, np.float32)
    W2 = np.asarray(W2, np.float32)

    def dup(a):
        return np.concatenate([a, a], axis=0).astype(np.float32)

    decay = np.float64(ALPHA) ** T
    n = np.arange(T - 1, -1, -1, dtype=np.float64)
    coeff = -THETA * (ALPHA ** (n + 1.0) - ETA ** (n + 1.0)) / (ALPHA - ETA)
    coeff_eff = (coeff * (2.0 / E) / B).astype(np.float32)
    # coeff_bc fm-packed: [p=e+64j, t'] = coeff_eff[t' + 512j]
    cb = np.zeros((128, 512), np.float32)
    cb[0:64, :] = coeff_eff[0:512][None, :]
    cb[64:128, :] = coeff_eff[512:1024][None, :]

    constsA = np.zeros((128, 384), np.float32)
    off = 0
    for w in [dup(Kw.T), dup(Vw.T), dup(Qw.T), dup(W1.T), dup(W2.T), dup(W2)]:
        constsA[:, off:off + 64] = w
        off += 64

    blobB = np.zeros((128, BLOBB_COLS), np.float32)
    blobB[:, 0:128] = np.eye(128, dtype=np.float32)
    blobB[:, 128:640] = cb
    blobB[0:64, 640:704] = (decay * W1.T).astype(np.float32)
    blobB[0:64, 704:768] = (decay * W2.T).astype(np.float32)

    in_maps = []
    for b in range(B):
        z = np.ascontiguousarray(x[b].T)  # (64, 1024)
        xfm = np.concatenate([z[:, :512], z[:, 512:]], axis=0)  # (128, 512)
        blobA = np.concatenate([xfm, constsA], axis=1)
        in_maps.append({"blobA": np.ascontiguousarray(blobA), "blobB": blobB})
    return in_maps


def _unpack(res_list):
    out = np.empty((B, T, E), np.float32)
    for b in range(B):
        o = res_list[b]["outp"]  # (128, 512)
        out[b] = np.concatenate([o[:64, :], o[64:, :]], axis=1).T
    return out


def run(inputs_dict, trace=False):
    nc = _get_nc()
    in_maps = _host_inputs(**inputs_dict)
    r = run_bass_kernel_spmd(nc, in_maps, list(range(B)), trace=trace)
    return _unpack(r.results), r


def kernel(x, Kw, Qw, Vw, W1, b1, W2, b2):
    out, _ = run(dict(x=x, Kw=Kw, Qw=Qw, Vw=Vw, W1=W1, b1=b1, W2=W2, b2=b2))
    return out


def bench(inputs_dict, n_lo=1000, n_hi=11000, reps=8):
    """Estimate per-body HW time via device-looped variants (includes the
    ~1-2us Tile loop back-edge, so an upper bound on single-shot time)."""
    import time
    in_maps = _host_inputs(**inputs_dict)
    times = {}
    for n in (n_lo, n_hi):
        nc = build_nc(bench_iters=n)
        run_bass_kernel_spmd(nc, in_maps, list(range(B)))  # compile+warm
        best = float("inf")
        for _ in range(reps):
            t0 = time.perf_counter()
            run_bass_kernel_spmd(nc, in_maps, list(range(B)))
            best = min(best, time.perf_counter() - t0)
        times[n] = best
    ns = (times[n_hi] - times[n_lo]) / (n_hi - n_lo) * 1e9
    return ns, times



# revision 9
# speedup vs baseline: 1.8600x; 1.8600x over previous
"""Trainium2 Bass kernel for the NeuralMemory (scatter_memory) problem.

Math (per batch b, closed form from the reference):
  keys/vals/q = l2norm_over_T(silu(x @ W.T))          (feature-major)
  a    = W1 @ keys^T ; h = silu(a) ; sp = silu'(a)
  cd   = coeff_eff[t] * ((W2 @ h^T) - vals^T)          (coeff_eff = coeff*2/(E*B))
  ce   = (W2^T @ cd) * sp
  W1f^T = decay*W1^T + sum_t keys[t] ce[t]^T           (T-contraction matmul)
  W2f^T = decay*W2^T + sum_t h[t] cd[t]^T
  b1f  = sum_t ce[t] ; b2f = sum_t cd[t]
  out  = W2f @ silu(W1f @ q^T + b1f) + b2f

Implementation notes:
  - bf16 operands for every matmul / transpose (1 cycle/row on PE vs 4 for
    fp32) and for most DVE elementwise work (2-4x perf modes).  PSUM
    accumulation stays fp32.
  - Single activation table (silu_and_others): Silu, Tanh, Square, Identity,
    Copy only -- no ACT table reloads.
  - silu'(a) built from tanh: sp = 0.5*((th+1)*(1-h) + 2h), th = tanh(a/2);
    the 0.5 folds into the 0.5*W2 stationary of the psE matmul.
  - vals subtraction fused into the psP matmul via a -I stationary.
  - tensor_tensor_reduce fuses the elementwise products with the l2norm
    sum-of-squares and the b1/b2 column sums.
  - rsqrt via fast-inverse-sqrt seed + 1 Newton step on DVE (batched k | vq).
  - Layout "fm" = feature-major packed (128, 512): partition p = e + 64*j,
    column t' with t = t' + 512*j.  One batch per NeuronCore (8 cores).
"""

import os

import numpy as np
import ml_dtypes

import concourse.bacc as bacc
import concourse.mybir as mybir
from concourse.tile import TileContext
from concourse.bass_utils import run_bass_kernel_spmd

ALPHA, ETA, THETA = 0.999, 0.6, 0.05
B, T, E, H = 8, 1024, 64, 64
FP = mybir.dt.float32
BF = mybir.dt.bfloat16
I32 = mybir.dt.int32
AF = mybir.ActivationFunctionType
ALU = mybir.AluOpType
MAGIC = 0x5F3759DF
BF_NP = ml_dtypes.bfloat16

_NC_CACHE = {}

# wts columns (bf16, all dup'd to 128 partitions):
#   kwT vwT qwT w1T w2T w2dh negI  (64 each)
WTS_COLS = 448


def _emit_dual(nc, psum, lhsT_dup, rhs_fm, start=True, stop=True):
    nc.tensor.matmul(psum[0:64, :], lhsT_dup[0:64, :], rhs_fm[0:64, :],
                     start=start, stop=stop)
    nc.tensor.matmul(psum[64:128, :], lhsT_dup[64:128, :], rhs_fm[64:128, :],
                     start=start, stop=stop)


def build_nc(finalize=True, bench_iters=1, ablate=()):
    ablate = set(ablate) | set(
        a for a in os.environ.get("KERNEL_ABLATE", "").split(",") if a)
    nc = bacc.Bacc("TRN2", target_bir_lowering=False, debug=False)

    xbf_d = nc.declare_dram_parameter("xbf", [128, 512], BF, isOutput=False)
    wts_d = nc.declare_dram_parameter("wts", [128, WTS_COLS], BF,
                                      isOutput=False)
    i128_d = nc.declare_dram_parameter("i128", [128, 128], BF, isOutput=False)
    coeff_d = nc.declare_dram_parameter("coeff", [128, 512], BF,
                                        isOutput=False)
    dw_d = nc.declare_dram_parameter("dw", [64, 128], FP, isOutput=False)
    out_d = nc.declare_dram_parameter("outp", [128, 512], FP, isOutput=True)

    with TileContext(nc) as tc:
        with (
            tc.tile_pool(name="persist", bufs=1) as pp,
            tc.tile_pool(name="rot", bufs=2) as rot,
            tc.tile_pool(name="small", bufs=1) as sm,
            tc.tile_pool(name="psmm", bufs=3, space="PSUM") as psmm,
            tc.tile_pool(name="pstr", bufs=3, space="PSUM") as pstr,
            tc.tile_pool(name="psacc", bufs=1, space="PSUM") as psacc,
        ):
            xbf = pp.tile([128, 512], BF, tag="xbf", name="xbf")
            nc.sync.dma_start(out=xbf[:, :], in_=xbf_d[:, :])
            wts = pp.tile([128, WTS_COLS], BF, tag="wts", name="wts")
            nc.sync.dma_start(out=wts[:, :], in_=wts_d[:, :])
            I128 = pp.tile([128, 128], BF, tag="i128", name="i128")
            nc.sync.dma_start(out=I128[:, :], in_=i128_d[:, :])
            coeff_bc = pp.tile([128, 512], BF, tag="coeff", name="coeff")
            nc.sync.dma_start(out=coeff_bc[:, :], in_=coeff_d[:, :])
            dw = pp.tile([64, 128], FP, tag="dw", name="dw")
            nc.sync.dma_start(out=dw[:, :], in_=dw_d[:, :])

            wt = {}
            for i, nm in enumerate(
                    ["kwT", "vwT", "qwT", "w1T", "w2T", "w2dh", "negI"]):
                wt[nm] = wts[:, 64 * i:64 * (i + 1)]
            dW1T = dw[0:64, 0:64]
            dW2T = dw[0:64, 64:128]

            # small constants (no DMA deps)
            magic = sm.tile([128, 2], I32, tag="magic", name="magic")
            nc.vector.memset(magic[:, :], MAGIC)
            wrow = pp.tile([128, 512], BF, tag="wrow", name="wrow")
            nc.gpsimd.memset(wrow[:, :], 0.0)
            warm_lhs = sm.tile([128, 1], BF, tag="warm_lhs", name="warm_lhs")
            nc.vector.memset(warm_lhs[:, :], 0.0)

            out_sb = pp.tile([128, 512], FP, tag="out_sb", name="out_sb")

            # ---- PE warm-up during the input DMA (ramps the p-state) ----
            pswarm = psmm.tile([128, 512], FP, tag="mm", name="mm")
            for _ in range(0 if "warm" in ablate else 8):
                nc.tensor.matmul(pswarm[0:1, :], warm_lhs[:, 0:1], wrow[:, :],
                                 start=True, stop=True)

            import contextlib
            _loop = contextlib.ExitStack()
            if bench_iters > 1:
                _loop.enter_context(tc.For_i(0, bench_iters, 1))

            def newton_rsqrt(s2, ncols, nm):
                """rs = 1/sqrt(s2), fast-inverse-sqrt seed + 1 Newton step.
                s2: [128, ncols] fp32 (both partition halves populated)."""
                sh1 = sm.tile([128, ncols], I32, tag=f"sh1_{nm}",
                              name=f"sh1_{nm}")
                nc.vector.tensor_scalar(
                    out=sh1[:, :], in0=s2[:, :].bitcast(I32), scalar1=1,
                    scalar2=None, op0=ALU.arith_shift_right)
                y0 = sm.tile([128, ncols], I32, tag=f"y0_{nm}",
                             name=f"y0_{nm}")
                nc.vector.tensor_sub(y0[:, :], magic[:, 0:ncols], sh1[:, :])
                y = y0[:, :].bitcast(FP)
                t = sm.tile([128, ncols], FP, tag=f"t_{nm}", name=f"t_{nm}")
                nc.vector.tensor_mul(t[:, :], y, y)
                t2 = sm.tile([128, ncols], FP, tag=f"t2_{nm}",
                             name=f"t2_{nm}")
                nc.vector.scalar_tensor_tensor(
                    out=t2[:, :], in0=t[:, :], scalar=0.5, in1=s2[:, :],
                    op0=ALU.mult, op1=ALU.mult)
                z = sm.tile([128, ncols], FP, tag=f"z_{nm}", name=f"z_{nm}")
                nc.vector.tensor_scalar(
                    out=z[:, :], in0=t2[:, :], scalar1=-1.0, scalar2=1.5,
                    op0=ALU.mult, op1=ALU.add)
                rs = sm.tile([128, ncols], FP, tag=f"rs_{nm}",
                             name=f"rs_{nm}")
                nc.vector.tensor_mul(rs[:, :], y, z[:, :])
                return rs

            # ================= phase 1: k/v/q streams =================
            psK = psmm.tile([128, 512], FP, tag="mm", name="mm")
            _emit_dual(nc, psK, wt["kwT"], xbf[:, :])
            psV = psmm.tile([128, 512], FP, tag="mm", name="mm")
            _emit_dual(nc, psV, wt["vwT"], xbf[:, :])
            psQ = psmm.tile([128, 512], FP, tag="mm", name="mm")
            _emit_dual(nc, psQ, wt["qwT"], xbf[:, :])

            silk = pp.tile([128, 512], BF, tag="silk", name="silk")
            nc.scalar.activation(silk[:, :], psK[:, :], AF.Silu)
            silv = pp.tile([128, 512], BF, tag="silv", name="silv")
            nc.scalar.activation(silv[:, :], psV[:, :], AF.Silu)
            silq = pp.tile([128, 512], BF, tag="silq", name="silq")
            nc.scalar.activation(silq[:, :], psQ[:, :], AF.Silu)

            # sum of squares: k on ACT (Square+accum, right after silu_k);
            # v/q on DVE (mul + reduce)
            def sumsq_dve(sil, nm):
                scr = rot.tile([128, 512], BF, tag="scr", name="scr")
                nc.vector.tensor_mul(scr[:, :], sil[:, :], sil[:, :])
                ss = sm.tile([128, 1], FP, tag=f"ss_{nm}", name=f"ss_{nm}")
                nc.vector.reduce_sum(ss[:, :], scr[:, :],
                                     axis=mybir.AxisListType.X)
                return ss

            scrk = rot.tile([128, 512], BF, tag="scr", name="scr")
            ssk = sm.tile([128, 1], FP, tag="ss_k", name="ss_k")
            nc.scalar.activation(scrk[:, :], silk[:, :], AF.Square,
                                 accum_out=ssk[:, :])
            # pack k: s2k[128,1] = (ssk_lo + ssk_hi) dup'd to both halves
            shk = sm.tile([64, 1], FP, tag="shk", name="shk")
            nc.vector.tensor_copy(shk[:, :], ssk[64:128, :])
            s2k = sm.tile([128, 1], FP, tag="s2k", name="s2k")
            nc.vector.tensor_add(s2k[0:64, :], ssk[0:64, :], shk[:, :])
            nc.vector.tensor_copy(s2k[64:128, :], s2k[0:64, :])
            rs_k = newton_rsqrt(s2k, 1, "k")

            # keys scale folds into the W1 stationary
            w1Ts = sm.tile([128, 64], BF, tag="w1Ts", name="w1Ts")
            nc.vector.tensor_scalar_mul(w1Ts[:, :], wt["w1T"], rs_k[:, :])

            ssv = sumsq_dve(silv, "v")
            ssq = sumsq_dve(silq, "q")
            # pack v|q: s2vq [128, 2]
            shv = sm.tile([64, 1], FP, tag="shv", name="shv")
            nc.vector.tensor_copy(shv[:, :], ssv[64:128, :])
            shq = sm.tile([64, 1], FP, tag="shq", name="shq")
            nc.vector.tensor_copy(shq[:, :], ssq[64:128, :])
            s2vq = sm.tile([128, 2], FP, tag="s2vq", name="s2vq")
            nc.vector.tensor_add(s2vq[0:64, 0:1], ssv[0:64, :], shv[:, :])
            nc.vector.tensor_add(s2vq[0:64, 1:2], ssq[0:64, :], shq[:, :])
            nc.vector.tensor_copy(s2vq[64:128, :], s2vq[0:64, :])
            rs_vq = newton_rsqrt(s2vq, 2, "vq")

            vals_fm = pp.tile([128, 512], BF, tag="vals_fm", name="vals_fm")
            nc.vector.tensor_scalar_mul(vals_fm[:, :], silv[:, :],
                                        rs_vq[:, 0:1])

            # ================= phase 2: a, h, th, sp2 =================
            psA = psmm.tile([128, 512], FP, tag="mm", name="mm")
            _emit_dual(nc, psA, w1Ts[:, :], silk[:, :])
            h_fm = pp.tile([128, 512], BF, tag="h_fm", name="h_fm")
            nc.scalar.activation(h_fm[:, :], psA[:, :], AF.Silu)
            th = rot.tile([128, 512], BF, tag="th", name="th")
            nc.scalar.activation(th[:, :], psA[:, :], AF.Tanh, scale=0.5)
            # silu'(a) = 0.5*((th+1)*(1-h) + 2h); the 0.5 lives in w2dh
            u1 = rot.tile([128, 512], BF, tag="u1", name="u1")
            nc.vector.tensor_scalar(out=u1[:, :], in0=h_fm[:, :],
                                    scalar1=-1.0, scalar2=1.0, op0=ALU.mult,
                                    op1=ALU.add)
            u2 = rot.tile([128, 512], BF, tag="u2", name="u2")
            nc.vector.scalar_tensor_tensor(
                out=u2[:, :], in0=th[:, :], scalar=1.0, in1=u1[:, :],
                op0=ALU.add, op1=ALU.mult)
            sp2 = pp.tile([128, 512], BF, tag="sp2", name="sp2")
            nc.vector.scalar_tensor_tensor(
                out=sp2[:, :], in0=h_fm[:, :], scalar=2.0, in1=u2[:, :],
                op0=ALU.mult, op1=ALU.add)

            # ================= phase 3: cd, ce (+ bias sums) ==========
            psP = psmm.tile([128, 512], FP, tag="mm", name="mm")
            _emit_dual(nc, psP, wt["w2T"], h_fm[:, :], start=True, stop=False)
            _emit_dual(nc, psP, wt["negI"], vals_fm[:, :], start=False,
                       stop=True)
            cd_fm = pp.tile([128, 512], BF, tag="cd_fm", name="cd_fm")
            nc.vector.tensor_mul(cd_fm[:, :], psP[:, :], coeff_bc[:, :])
            # b2f = sum_t cd on ACT (Identity + accum; off critical path)
            scrb = rot.tile([128, 512], BF, tag="scr", name="scr")
            b2s = sm.tile([128, 1], FP, tag="b2s", name="b2s")
            nc.scalar.activation(scrb[:, :], cd_fm[:, :], AF.Identity,
                                 accum_out=b2s[:, :])

            psE = psmm.tile([128, 512], FP, tag="mm", name="mm")
            _emit_dual(nc, psE, wt["w2dh"], cd_fm[:, :])
            ce_fm = pp.tile([128, 512], BF, tag="ce_fm", name="ce_fm")
            nc.vector.tensor_mul(ce_fm[:, :], psE[:, :], sp2[:, :])
            b1s = sm.tile([128, 1], FP, tag="b1s", name="b1s")
            nc.vector.reduce_sum(b1s[:, :], ce_fm[:, :],
                                 axis=mybir.AxisListType.X)

            # bias columns finalize on Pool (idle engine)
            def bias_col(bs, nm):
                bh = sm.tile([64, 1], FP, tag=f"bh_{nm}", name=f"bh_{nm}")
                nc.gpsimd.tensor_copy(bh[:, :], bs[64:128, :])
                bc = sm.tile([128, 1], FP, tag=f"bc_{nm}", name=f"bc_{nm}")
                nc.gpsimd.tensor_add(bc[0:64, :], bs[0:64, :], bh[:, :])
                nc.gpsimd.tensor_copy(bc[64:128, :], bc[0:64, :])
                return bc

            b2c = bias_col(b2s, "b2")
            b1c = bias_col(b1s, "b1")

            # ============ phase 4: transposes to T-major chunks ========
            # transpose group: 4 chunks of a (128,512) bf16 fm tensor into one
            # (128,512) bf16 PSUM tile; one batched copy back to SBUF.
            def tr_group(src_fm, nm):
                ps = pstr.tile([128, 512], BF, tag="tr", name="tr")
                for c in range(4):
                    nc.tensor.transpose(ps[:, 128 * c:128 * (c + 1)],
                                        src_fm[:, 128 * c:128 * (c + 1)],
                                        I128[:, :])
                return ps

            # k/h transposes early; copies on Pool (idle); cd on ACT, ce on
            # DVE (latency-critical tail)
            psTk = tr_group(silk, "k")
            ktr = pp.tile([128, 512], BF, tag="ktr", name="ktr")
            nc.vector.tensor_copy(ktr[:, :], psTk[:, :])

            psTh = tr_group(h_fm, "h")
            htr = pp.tile([128, 512], BF, tag="htr", name="htr")
            nc.scalar.copy(htr[:, :], psTh[:, :])

            psTd = tr_group(cd_fm, "d")
            dtr = pp.tile([128, 512], BF, tag="dtr", name="dtr")
            nc.scalar.copy(dtr[:, :], psTd[:, :])

            psTe = tr_group(ce_fm, "e")
            etr = pp.tile([128, 512], BF, tag="etr", name="etr")
            nc.vector.tensor_copy(etr[:, :], psTe[:, :])

            # ============ phase 5: T-contraction ======================
            # Q22 = sum_t h[t] cd[t]^T ; Q11' = sum_t silk[t] ce[t]^T
            psB2 = psacc.tile([128, 64], FP, tag="psB2", name="psB2")
            for cc in range(8):
                c, j = cc % 4, cc // 4
                sl = slice(128 * c + 64 * j, 128 * c + 64 * j + 64)
                nc.tensor.matmul(psB2[64:128, :], htr[:, sl], dtr[:, sl],
                                 start=(cc == 0), stop=(cc == 7),
                                 skip_group_check=True)
            psB1 = psacc.tile([64, 64], FP, tag="psB1", name="psB1")
            for cc in range(8):
                c, j = cc % 4, cc // 4
                sl = slice(128 * c + 64 * j, 128 * c + 64 * j + 64)
                nc.tensor.matmul(psB1[:, :], ktr[:, sl], etr[:, sl],
                                 start=(cc == 0), stop=(cc == 7),
                                 skip_group_check=True)

            # ============ phase 6: final fast weights =================
            # W1fT = (rs_k*rs_q)[e]*Q11' + rs_q[e]*decay*W1T
            # W2fT = Q22 + decay*W2T
            q11 = psB1[:, :]
            q22 = psB2[64:128, :]
            w2fT = pp.tile([128, 64], BF, tag="w2fT", name="w2fT")
            nc.vector.scalar_tensor_tensor(
                out=w2fT[0:64, :], in0=q22, scalar=1.0, in1=dW2T,
                op0=ALU.mult, op1=ALU.add)
            nc.vector.scalar_tensor_tensor(
                out=w2fT[64:128, :], in0=q22, scalar=1.0, in1=dW2T,
                op0=ALU.mult, op1=ALU.add)

            skq = sm.tile([64, 1], FP, tag="skq", name="skq")
            nc.vector.tensor_mul(skq[:, :], rs_k[0:64, :], rs_vq[0:64, 1:2])
            dW1q = sm.tile([64, 64], FP, tag="dW1q", name="dW1q")
            nc.vector.tensor_scalar_mul(dW1q[:, :], dW1T, rs_vq[0:64, 1:2])
            w1fT = pp.tile([128, 64], BF, tag="w1fT", name="w1fT")
            nc.vector.scalar_tensor_tensor(
                out=w1fT[0:64, :], in0=q11, scalar=skq[:, :], in1=dW1q[:, :],
                op0=ALU.mult, op1=ALU.add)
            nc.vector.scalar_tensor_tensor(
                out=w1fT[64:128, :], in0=q11, scalar=skq[:, :],
                in1=dW1q[:, :], op0=ALU.mult, op1=ALU.add)

            # ============ phase 7: retrieval ==========================
            psR1 = psmm.tile([128, 512], FP, tag="mm", name="mm")
            _emit_dual(nc, psR1, w1fT[:, :], silq[:, :])
            h2_fm = pp.tile([128, 512], BF, tag="h2_fm", name="h2_fm")
            nc.scalar.activation(h2_fm[:, :], psR1[:, :], AF.Silu,
                                 bias=b1c[:, :])
            psR2 = psmm.tile([128, 512], FP, tag="mm", name="mm")
            _emit_dual(nc, psR2, w2fT[:, :], h2_fm[:, :])
            nc.scalar.activation(out_sb[:, :], psR2[:, :], AF.Identity,
                                 bias=b2c[:, :])

            _loop.close()
            nc.sync.dma_start(out=out_d[:, :], in_=out_sb[:, :])

    if finalize:
        nc.finalize()
    return nc


def _get_nc():
    if "nc" not in _NC_CACHE:
        _NC_CACHE["nc"] = build_nc()
    return _NC_CACHE["nc"]


def _host_inputs(x, Kw, Qw, Vw, W1, b1, W2, b2):
    x = np.asarray(x, np.float32)
    Kw = np.asarray(Kw, np.float32)
    Qw = np.asarray(Qw, np.float32)
    Vw = np.asarray(Vw, np.float32)
    W1 = np.asarray(W1, np.float32)
    W2 = np.asarray(W2, np.float32)

    def dup(a):
        return np.concatenate([a, a], axis=0)

    decay = np.float64(ALPHA) ** T
    n = np.arange(T - 1, -1, -1, dtype=np.float64)
    coeff = -THETA * (ALPHA ** (n + 1.0) - ETA ** (n + 1.0)) / (ALPHA - ETA)
    coeff_eff = (coeff * (2.0 / E) / B).astype(np.float32)
    # coeff_bc fm-packed: [p=e+64j, t'] = coeff_eff[t' + 512j]
    cb = np.zeros((128, 512), np.float32)
    cb[0:64, :] = coeff_eff[0:512][None, :]
    cb[64:128, :] = coeff_eff[512:1024][None, :]

    wts = np.zeros((128, WTS_COLS), np.float32)
    mats = [Kw.T, Vw.T, Qw.T, W1.T, W2.T, 0.5 * W2,
            -np.eye(64, dtype=np.float32)]
    for i, w in enumerate(mats):
        wts[:, 64 * i:64 * (i + 1)] = dup(w)

    dwb = np.zeros((64, 128), np.float32)
    dwb[:, 0:64] = (decay * W1.T).astype(np.float32)
    dwb[:, 64:128] = (decay * W2.T).astype(np.float32)

    wts_bf = wts.astype(BF_NP)
    i128_bf = np.eye(128, dtype=np.float32).astype(BF_NP)
    cb_bf = cb.astype(BF_NP)

    in_maps = []
    for b_i in range(B):
        z = np.ascontiguousarray(x[b_i].T)  # (64, 1024)
        xfm = np.concatenate([z[:, :512], z[:, 512:]], axis=0)  # (128, 512)
        in_maps.append({
            "xbf": np.ascontiguousarray(xfm.astype(BF_NP)),
            "wts": wts_bf,
            "i128": i128_bf,
            "coeff": cb_bf,
            "dw": dwb,
        })
    return in_maps


def _unpack(res_list):
    out = np.empty((B, T, E), np.float32)
    for b_i in range(B):
        o = res_list[b_i]["outp"]  # (128, 512)
        out[b_i] = np.concatenate([o[:64, :], o[64:, :]], axis=1).T
    return out


def run(inputs_dict, trace=False):
    nc = _get_nc()
    in_maps = _host_inputs(**inputs_dict)
    r = run_bass_kernel_spmd(nc, in_maps, list(range(B)), trace=trace)
    return _unpack(r.results), r


def kernel(x, Kw, Qw, Vw, W1, b1, W2, b2):
    out, _ = run(dict(x=x, Kw=Kw, Qw=Qw, Vw=Vw, W1=W1, b1=b1, W2=W2, b2=b2))
    return out


def bench(inputs_dict, n_lo=1000, n_hi=11000, reps=8):
    """Estimate per-body HW time via device-looped variants (includes the
    ~1-2us Tile loop back-edge, so an upper bound on single-shot time)."""
    import time
    in_maps = _host_inputs(**inputs_dict)
    times = {}
    for n in (n_lo, n_hi):
        nc = build_nc(bench_iters=n)
        run_bass_kernel_spmd(nc, in_maps, list(range(B)))  # compile+warm
        best = float("inf")
        for _ in range(reps):
            t0 = time.perf_counter()
            run_bass_kernel_spmd(nc, in_maps, list(range(B)))
            best = min(best, time.perf_counter() - t0)
        times[n] = best
    ns = (times[n_hi] - times[n_lo]) / (n_hi - n_lo) * 1e9
    return ns, times
